# revision 22
# baseline (speedup 1.0000x reference)
# Multi-head attention (softmax_plus variant) on 8 Trainium2 NeuronCores.
#
# Problem (hardcoded): B=2, S=2048, D=768, HEADS=12, KEY_SIZE=HEAD_SIZE=64,
# OUT_DIM=768.  out = softmax_plus((q Wq)(k Wk)^T / 8, mask) (v Wv) Wo + bo
#
# Sharding: batch x head-group.  Core c -> batch b=c//4, heads 3g..3g+2 with
# g=c%4.  Wq/Wk/Wv column-sharded, Wo row-sharded; each core returns a
# (2048, 768) partial of its batch's output; host sums groups of 4 + bias.
#
# Device layout notes:
#  - q/k/v are fed pre-transposed (D, S) so the contraction dim (din) sits on
#    SBUF partitions for the projection matmuls.
#  - Projections are computed transposed: qwT/kwT (64h, 2048s).  Scores are
#    computed transposed too: S_T[k, q], so keys sit on partitions and the
#    sequence mask becomes a per-partition bias folded into the Exp
#    activation (exp(S + maskbias), maskbias = -1e12 on masked keys -> p=0).
#  - softmax_plus scale log(l)/log(512)/sqrt(64) is folded into Wq host-side.
#  - vw is needed keys-on-partitions for A@V; computed transposed then
#    PE-transposed back, with a ones column appended per head so the A@V
#    matmul also produces the softmax denominator (row 64 of ctx PSUM).
#  - q/k-side matmuls run as float32r (full PE rate at N>=512, ~12-bit
#    mantissa).  The v side (v, Wv, p=exp(S)) optionally runs in bf16, which
#    only perturbs the attention-weighted average, not the logits.
#  - Schedule: q,k are loaded/projected first and the first head's score/exp
#    block is emitted before the v pipeline, so ScalarE (the bottleneck
#    engine) starts as soon as kwT exists.  Attention is q-slab-major
#    (2 slabs x 1024 queries); each head is a dense S/exp block followed by a
#    dense A@V block (deep p buffer decouples ScalarE from PE), and each
#    slab's output projection is interleaved into the next slab's attention.

import os
import sys

import numpy as np

for _p in ("/opt/trn_rl_repo", "/root/.axon_site/_ro/trn_rl_repo"):
    if os.path.isdir(_p) and _p not in sys.path:
        sys.path.append(_p)

import ml_dtypes
import concourse.bass as bass
import concourse.tile as tile
from concourse import bacc, mybir
from concourse.bass_utils import run_bass_kernel_spmd
from concourse.masks import make_identity

F32 = mybir.dt.float32
F32R = mybir.dt.float32r
BF16 = mybir.dt.bfloat16

QK_BF16 = False  # feed q/k (and Wq/Wk) as bf16: halves their DMA, logits lose
                 # ~3 mantissa bits (exp-amplified) -> only flip if error OK
V_BF16 = True    # v-side (v, Wv, vw, p) in bf16: halves v DMA, perturbs only
                 # the attention average (~0.2% typ)

B, S, D = 2, 2048, 768
HEADS, KS = 12, 64  # total heads, key/head size
HPC = 3  # heads per core
HD = HPC * KS  # 192 head dims per core
N_CORES = 8
MASK_VALUE = -1e12
LOG_512 = float(np.log(512.0))

KC = D // 128  # 6 contraction chunks for projections
KT = S // 128  # 16 key tiles
QH = 1024      # query slab width
NSLAB = S // QH

QK_DT = BF16 if QK_BF16 else F32R
V_DT = BF16 if V_BF16 else F32R
VW_DT = BF16 if V_BF16 else F32
NPBF = ml_dtypes.bfloat16


def build_program():
    nc = bacc.Bacc("TRN2", target_bir_lowering=False, debug=False)

    qT = nc.dram_tensor("qT", [D, S], QK_DT, kind="ExternalInput").ap()
    kT = nc.dram_tensor("kT", [D, S], QK_DT, kind="ExternalInput").ap()
    vT = nc.dram_tensor("vT", [D, S], V_DT, kind="ExternalInput").ap()
    wq = nc.dram_tensor("wq", [D, HD], QK_DT, kind="ExternalInput").ap()
    wk = nc.dram_tensor("wk", [D, HD], QK_DT, kind="ExternalInput").ap()
    wv = nc.dram_tensor("wv", [D, HD], V_DT, kind="ExternalInput").ap()
    wo = nc.dram_tensor("wo", [HD, D], F32R, kind="ExternalInput").ap()
    bqv = nc.dram_tensor("bqv", [HD, 1], F32, kind="ExternalInput").ap()
    bkv = nc.dram_tensor("bkv", [HD, 1], F32, kind="ExternalInput").ap()
    bvv = nc.dram_tensor("bvv", [HD, 1], F32, kind="ExternalInput").ap()
    mb = nc.dram_tensor("mb", [128, KT], F32, kind="ExternalInput").ap()
    onec = nc.dram_tensor("onec", [1, 1], VW_DT, kind="ExternalInput").ap()
    out = nc.dram_tensor("out", [S, D], F32, kind="ExternalOutput").ap()

    with tile.TileContext(nc) as tc:
        consts = tc.alloc_tile_pool(name="consts", bufs=1)
        # weights as (128, KC, HD): partition=din%128, free=(din//128, dout)
        w_sb = {}
        for name, dram, dt_ in (("wv", wv, V_DT), ("wq", wq, QK_DT),
                                ("wk", wk, QK_DT)):
            t = consts.tile([128, KC, HD], dt_, tag=name)
            nc.sync.dma_start(out=t, in_=dram.rearrange("(c p) d -> p c d", p=128))
            w_sb[name] = t
        bias_sb = {}
        for name, dram in (("bq", bqv), ("bk", bkv), ("bv", bvv)):
            tA = consts.tile([128, 1], F32, tag=name + "A")
            nc.sync.dma_start(out=tA, in_=dram[0:128, :])
            tB = consts.tile([KS, 1], F32, tag=name + "B")
            nc.sync.dma_start(out=tB, in_=dram[128:HD, :])
            bias_sb[name] = (tA, tB)
        mb_sb = consts.tile([128, KT], F32, tag="mb")
        nc.sync.dma_start(out=mb_sb, in_=mb)
        woA = consts.tile([128, D], F32R, tag="woA")
        nc.sync.dma_start(out=woA, in_=wo[0:128, :])
        woB = consts.tile([KS, D], F32R, tag="woB")
        nc.sync.dma_start(out=woB, in_=wo[128:HD, :])
        ident = consts.tile([128, 128], VW_DT, tag="ident")
        make_identity(nc, ident)

        # PE warm-up: dummy back-to-back matmuls so the HAM clock-gate sees a
        # busy PE while the first input chunks are still in flight; their
        # results are never read.
        with tc.tile_pool(name="warm_ps", bufs=1, space="PSUM") as wp:
            warm = wp.tile([128, 128], F32, tag="warm")
            for _ in range(24):
                nc.tensor.matmul(warm, ident, ident, start=True, stop=True)

        # persistent SBUF holding projected tensors
        proj = tc.alloc_tile_pool(name="proj", bufs=1)
        qA = proj.tile([128, S], F32R, tag="qA")  # heads 0,1 of qwT (scaled)
        qB = proj.tile([KS, S], F32R, tag="qB")  # head 2
        kA = proj.tile([128, S], F32R, tag="kA")
        kB = proj.tile([KS, S], F32R, tag="kB")
        # vw_ext[p, t, h, 0:64] = vw[t*128+p, 64h : 64h+64]; [..., 64] = 1.0
        vw_ext = proj.tile([128, KT, HPC, KS + 1], VW_DT, tag="vw")
        ones_bcast = bass.AP(
            tensor=onec.tensor, offset=0,
            ap=[[0, 128], [0, KT * HPC], [1, 1]],
        )
        nc.sync.dma_start(out=vw_ext[:, :, :, KS:KS + 1], in_=ones_bcast)
        ctxA = proj.tile([128, S], F32R, tag="ctxA")  # heads 0,1 stacked
        ctxB = proj.tile([KS, S], F32R, tag="ctxB")   # head 2

        def head_slices(tileA, tileB):
            return [tileA[0:KS], tileA[KS:128], tileB[0:KS]]

        q_heads = head_slices(qA, qB)
        k_heads = head_slices(kA, kB)

        # ---- v pipeline first: with the v side in bf16 its DMA is small, and
        # doing it before q/k keeps PE order conflict-free (attention needs
        # vw_ext from the very first A@V).
        vwTA = proj.tile([128, S], VW_DT, tag="vwTA")
        vwTB = proj.tile([KS, S], VW_DT, tag="vwTB")
        # single input pool for v AND q/k tiles, allocated up front so the
        # q/k input DMAs never wait on v-phase compute via address reuse
        xin = tc.alloc_tile_pool(name="xin", bufs=6)
        bA, bB = bias_sb["bv"]
        with tc.tile_pool(name="vproj_ps", bufs=1, space="PSUM") as pp:
            psA = pp.tile([128, S], F32, tag="pA")
            psB = pp.tile([KS, S], F32, tag="pB")
            for c in range(KC):
                xt = xin.tile([128, S], V_DT, tag="xv", bufs=6)
                nc.sync.dma_start(out=xt, in_=vT[c * 128:(c + 1) * 128, :])
                for n in range(S // 512):
                    ns = slice(n * 512, (n + 1) * 512)
                    nc.tensor.matmul(
                        psA[:, ns], w_sb["wv"][:, c, 0:128], xt[:, ns],
                        start=(c == 0), stop=(c == KC - 1),
                    )
                for n in range(S // 512):
                    ns = slice(n * 512, (n + 1) * 512)
                    nc.tensor.matmul(
                        psB[:, ns], w_sb["wv"][:, c, 128:HD], xt[:, ns],
                        start=(c == 0), stop=(c == KC - 1),
                    )
            nc.vector.tensor_scalar_add(vwTA, psA, bA)
            nc.vector.tensor_scalar_add(vwTB, psB, bB)

        with tc.tile_pool(name="tr_ps", bufs=2, space="PSUM") as trp:
            for t in range(KT):
                ts_ = slice(t * 128, (t + 1) * 128)
                trA = trp.tile([128, 128], VW_DT, tag="tr")
                nc.tensor.transpose(trA, vwTA[:, ts_], ident)
                nc.vector.tensor_copy(
                    out=vw_ext[:, t, 0:2, 0:KS],
                    in_=trA.rearrange("p (h x) -> p h x", h=2),
                )
                trB = trp.tile([128, KS], VW_DT, tag="tr")
                nc.tensor.transpose(trB, vwTB[:, ts_], ident[0:KS, 0:KS])
                nc.vector.tensor_copy(out=vw_ext[:, t, 2, 0:KS], in_=trB)
        # ---- q/k projection: x (D,S) -> xwT (HD, S), full-width PSUM
        def project_qk(xT_dram, w_tile, biasAB, outA, outB, xin_pool, pp):
            psA = pp.tile([128, S], F32, tag="pA")
            psB = pp.tile([KS, S], F32, tag="pB")
            for c in range(KC):
                xt = xin_pool.tile([128, S], QK_DT, tag="x", bufs=8)
                nc.sync.dma_start(out=xt, in_=xT_dram[c * 128:(c + 1) * 128, :])
                for n in range(S // 512):
                    ns = slice(n * 512, (n + 1) * 512)
                    nc.tensor.matmul(
                        psA[:, ns], w_tile[:, c, 0:128], xt[:, ns],
                        start=(c == 0), stop=(c == KC - 1),
                    )
                for n in range(S // 512):
                    ns = slice(n * 512, (n + 1) * 512)
                    nc.tensor.matmul(
                        psB[:, ns], w_tile[:, c, 128:HD], xt[:, ns],
                        start=(c == 0), stop=(c == KC - 1),
                    )
            bA, bB = biasAB
            nc.vector.tensor_scalar_add(outA, psA, bA)
            nc.vector.tensor_scalar_add(outB, psB, bB)

        with tc.tile_pool(name="qkproj_ps", bufs=1, space="PSUM") as pp:
            project_qk(qT, w_sb["wq"], bias_sb["bq"], qA, qB, xin, pp)
            project_qk(kT, w_sb["wk"], bias_sb["bk"], kA, kB, xin, pp)
        xin.release()

        # ---- attention building blocks
        small = tc.alloc_tile_pool(name="small", bufs=1)
        psb = tc.alloc_tile_pool(name="p_sb", bufs=34)
        ob = tc.alloc_tile_pool(name="o_sb", bufs=3)
        sp = tc.alloc_tile_pool(name="s_ps", bufs=2, space="PSUM")

        def sexp_block(q0, h):
            """Scores + exp for one (slab, head): 16 key tiles -> p tiles."""
            qh_t, kh_t = q_heads[h], k_heads[h]
            p_tiles = []
            for t in range(KT):
                lhs_k = kh_t[:, t * 128:(t + 1) * 128]
                s_ps = sp.tile([128, QH], F32, tag="s")
                for j in range(2):
                    qs = slice(q0 + j * 512, q0 + (j + 1) * 512)
                    nc.tensor.matmul(
                        s_ps[:, j * 512:(j + 1) * 512], lhs_k, qh_t[:, qs],
                        start=True, stop=True,
                    )
                p_t = psb.tile([128, QH], VW_DT, tag="p")
                nc.scalar.activation(
                    out=p_t, in_=s_ps,
                    func=mybir.ActivationFunctionType.Exp,
                    bias=mb_sb[:, t:t + 1], scale=1.0,
                )
                p_tiles.append(p_t)
            return p_tiles

        # ---- main attention + interleaved output projection
        with tc.tile_pool(name="ctx_ps", bufs=2, space="PSUM") as ctxp:

            def av_block(q0, h, p_tiles):
                """A@V (+ denominator) accumulation and normalization."""
                ctx_ps = ctxp.tile([KS + 1, QH], F32, tag="ctx")
                for t in range(KT):
                    for j in range(2):
                        nc.tensor.matmul(
                            ctx_ps[:, j * 512:(j + 1) * 512],
                            vw_ext[:, t, h, :],
                            p_tiles[t][:, j * 512:(j + 1) * 512],
                            start=(t == 0), stop=(t == KT - 1),
                        )
                recip = small.tile([1, QH], F32, tag="recip")
                nc.vector.reciprocal(recip, ctx_ps[KS:KS + 1, :])
                rb = small.tile([KS, QH], F32, tag="rb")
                nc.gpsimd.partition_broadcast(rb, recip)
                dst = (ctxA[h * KS:(h + 1) * KS, q0:q0 + QH] if h < 2
                       else ctxB[:, q0:q0 + QH])
                nc.vector.tensor_mul(dst, ctx_ps[0:KS, :], rb)

            def out_proj(qt):
                # out[qt*128 : +128, :] = sum_h ctx[h, qt cols].T @ wo[h]
                # (borrows a (65, 1024) "ctx" PSUM slot)
                o_ps = ctxp.tile([128, D], F32, tag="ctx")
                qs = slice(qt * 128, (qt + 1) * 128)
                for noff, nsz in ((0, 512), (512, 256)):
                    nc.tensor.matmul(
                        o_ps[:, noff:noff + nsz],
                        ctxA[:, qs], woA[:, noff:noff + nsz],
                        start=True, stop=False,
                    )
                    nc.tensor.matmul(
                        o_ps[:, noff:noff + nsz],
                        ctxB[:, qs], woB[:, noff:noff + nsz],
                        start=False, stop=True,
                    )
                o_sb = ob.tile([128, D], F32, tag="o")
                nc.vector.tensor_copy(o_sb, o_ps)
                nc.sync.dma_start(out=out[qs, :], in_=o_sb)

            TPS = QH // 128  # out-proj tiles per slab
            pending = []
            for half in range(NSLAB):
                q0 = half * QH
                p0 = sexp_block(q0, 0)
                p1 = sexp_block(q0, 1)
                av_block(q0, 0, p0)
                for qt in pending[:TPS // 2]:
                    out_proj(qt)
                p2 = sexp_block(q0, 2)
                av_block(q0, 1, p1)
                for qt in pending[TPS // 2:]:
                    out_proj(qt)
                av_block(q0, 2, p2)
                pending = list(range(half * TPS, (half + 1) * TPS))
            for qt in pending:
                out_proj(qt)

        sp.release()
        ob.release()
        psb.release()
        small.release()
        proj.release()
        consts.release()

    nc.compile()
    return nc


_NC_CACHE = []


def _get_nc():
    if not _NC_CACHE:
        _NC_CACHE.append(build_program())
    return _NC_CACHE[0]


def _qk(a):
    return a.astype(NPBF) if QK_BF16 else a


def _vv(a):
    return a.astype(NPBF) if V_BF16 else a


def _prep_core_inputs(q, k, v, Wq, bq, Wk, bk, Wv, bv, Wo, v_mask):
    """Build the 8 per-core input maps (host-side sharding + layout)."""
    f = np.float32
    in_maps = []
    for b in range(B):
        l = max(float(np.asarray(v_mask[b], dtype=np.int64).sum()), 1.0)
        cb = np.log(l) / LOG_512 / np.sqrt(float(KS))
        qTb = _qk(np.ascontiguousarray(np.asarray(q[b], f).T))
        kTb = _qk(np.ascontiguousarray(np.asarray(k[b], f).T))
        vTb = _vv(np.ascontiguousarray(np.asarray(v[b], f).T))
        mbias = ((1.0 - np.asarray(v_mask[b], f)) * MASK_VALUE).astype(f)
        mb_t = np.ascontiguousarray(mbias.reshape(KT, 128).T)
        for g in range(4):
            sl = slice(g * HD, (g + 1) * HD)
            in_maps.append({
                "qT": qTb,
                "kT": kTb,
                "vT": vTb,
                "wq": _qk(np.ascontiguousarray(np.asarray(Wq, f)[:, sl] * cb)),
                "wk": _qk(np.ascontiguousarray(np.asarray(Wk, f)[:, sl])),
                "wv": _vv(np.ascontiguousarray(np.asarray(Wv, f)[:, sl])),
                "wo": np.ascontiguousarray(np.asarray(Wo, f)[sl, :]),
                "bqv": (np.asarray(bq, f)[sl] * cb).reshape(HD, 1).copy(),
                "bkv": np.asarray(bk, f)[sl].reshape(HD, 1).copy(),
                "bvv": np.asarray(bv, f)[sl].reshape(HD, 1).copy(),
                "mb": mb_t,
                "onec": np.ones((1, 1), NPBF if V_BF16 else f),
            })
    return in_maps


def kernel(q, k, v, Wq, bq, Wk, bk, Wv, bv, Wo, bo, v_mask, **_unused):
    nc = _get_nc()
    in_maps = _prep_core_inputs(q, k, v, Wq, bq, Wk, bk, Wv, bv, Wo, v_mask)
    res = run_bass_kernel_spmd(nc, in_maps, core_ids=list(range(N_CORES)))
    outs = [r["out"] for r in res.results]
    full = np.empty((B, S, D), np.float32)
    bo_f = np.asarray(bo, np.float32)
    for b in range(B):
        acc = outs[4 * b].copy()
        for g in range(1, 4):
            acc += outs[4 * b + g]
        full[b] = acc + bo_f
    return full


if __name__ == "__main__":
    np.random.seed(0)
    q = np.random.randn(B, S, D).astype(np.float32)
    k = np.random.randn(B, S, D).astype(np.float32)
    v = np.random.randn(B, S, D).astype(np.float32)
    Wq = np.random.randn(D, HEADS * KS).astype(np.float32) * 0.06
    Wk = np.random.randn(D, HEADS * KS).astype(np.float32) * 0.06
    Wv = np.random.randn(D, HEADS * KS).astype(np.float32) * 0.06
    Wo = np.random.randn(HEADS * KS, D).astype(np.float32) * 0.06
    z = np.zeros(HEADS * KS, np.float32)
    bo = np.zeros(D, np.float32)
    mask = np.random.randint(0, 2, (B, S)).astype(np.int32)
    o = kernel(q=q, k=k, v=v, Wq=Wq, bq=z, Wk=Wk, bk=z, Wv=Wv, bv=z,
               Wo=Wo, bo=bo, v_mask=mask)
    print(o.shape, o.dtype, float(np.abs(o).max()))


# revision 28
# speedup vs baseline: 1.0096x; 1.0096x over previous
# Multi-head attention (softmax_plus variant) on 8 Trainium2 NeuronCores.
#
# Problem (hardcoded): B=2, S=2048, D=768, HEADS=12, KEY_SIZE=HEAD_SIZE=64,
# OUT_DIM=768.  out = softmax_plus((q Wq)(k Wk)^T / 8, mask) (v Wv) Wo + bo
#
# Sharding: batch x head-group.  Core c -> batch b=c//4, heads 3g..3g+2 with
# g=c%4.  Wq/Wk/Wv column-sharded, Wo row-sharded; each core returns a
# (2048, 768) partial of its batch's output; host sums groups of 4 + bias.
#
# Device layout notes:
#  - q/k/v are fed pre-transposed (D, S) so the contraction dim (din) sits on
#    SBUF partitions for the projection matmuls.
#  - Projections are computed transposed: qwT/kwT (64h, 2048s).  Scores are
#    computed transposed too: S_T[k, q], so keys sit on partitions and the
#    sequence mask becomes a per-partition bias folded into the Exp
#    activation (exp(S + maskbias), maskbias = -1e12 on masked keys -> p=0).
#  - softmax_plus scale log(l)/log(512)/sqrt(64) is folded into Wq host-side.
#  - vw is needed keys-on-partitions for A@V; computed transposed then
#    PE-transposed back, with a ones column appended per head so the A@V
#    matmul also produces the softmax denominator (row 64 of ctx PSUM).
#  - q/k-side matmuls run as float32r (full PE rate at N>=512, ~12-bit
#    mantissa).  The v side (v, Wv, p=exp(S)) optionally runs in bf16, which
#    only perturbs the attention-weighted average, not the logits.
#  - Schedule: q,k are loaded/projected first and the first head's score/exp
#    block is emitted before the v pipeline, so ScalarE (the bottleneck
#    engine) starts as soon as kwT exists.  Attention is q-slab-major
#    (2 slabs x 1024 queries); each head is a dense S/exp block followed by a
#    dense A@V block (deep p buffer decouples ScalarE from PE), and each
#    slab's output projection is interleaved into the next slab's attention.

import os
import sys

import numpy as np

for _p in ("/opt/trn_rl_repo", "/root/.axon_site/_ro/trn_rl_repo"):
    if os.path.isdir(_p) and _p not in sys.path:
        sys.path.append(_p)

import ml_dtypes
import concourse.bass as bass
import concourse.tile as tile
from concourse import bacc, mybir
from concourse.bass_utils import run_bass_kernel_spmd
from concourse.masks import make_identity

F32 = mybir.dt.float32
F32R = mybir.dt.float32r
BF16 = mybir.dt.bfloat16

QK_BF16 = False  # feed q/k (and Wq/Wk) as bf16: halves their DMA, logits lose
                 # ~3 mantissa bits (exp-amplified) -> only flip if error OK
V_BF16 = True    # v-side (v, Wv, vw, p) in bf16: halves v DMA, perturbs only
                 # the attention average (~0.2% typ)

B, S, D = 2, 2048, 768
HEADS, KS = 12, 64  # total heads, key/head size
HPC = 3  # heads per core
HD = HPC * KS  # 192 head dims per core
N_CORES = 8
MASK_VALUE = -1e12
LOG_512 = float(np.log(512.0))

KC = D // 128  # 6 contraction chunks for projections
KT = S // 128  # 16 key tiles
QH = 1024      # query slab width
NSLAB = S // QH

QK_DT = BF16 if QK_BF16 else F32R
V_DT = BF16 if V_BF16 else F32R
VW_DT = BF16 if V_BF16 else F32
NPBF = ml_dtypes.bfloat16


def build_program():
    nc = bacc.Bacc("TRN2", target_bir_lowering=False, debug=False)

    qT = nc.dram_tensor("qT", [D, S], QK_DT, kind="ExternalInput").ap()
    kT = nc.dram_tensor("kT", [D, S], QK_DT, kind="ExternalInput").ap()
    vT = nc.dram_tensor("vT", [D, S], V_DT, kind="ExternalInput").ap()
    wq = nc.dram_tensor("wq", [D, HD], QK_DT, kind="ExternalInput").ap()
    wk = nc.dram_tensor("wk", [D, HD], QK_DT, kind="ExternalInput").ap()
    wv = nc.dram_tensor("wv", [D, HD], V_DT, kind="ExternalInput").ap()
    wo = nc.dram_tensor("wo", [HD, D], F32R, kind="ExternalInput").ap()
    bqv = nc.dram_tensor("bqv", [HD, 1], F32, kind="ExternalInput").ap()
    bkv = nc.dram_tensor("bkv", [HD, 1], F32, kind="ExternalInput").ap()
    bvv = nc.dram_tensor("bvv", [HD, 1], F32, kind="ExternalInput").ap()
    mb = nc.dram_tensor("mb", [128, KT], F32, kind="ExternalInput").ap()
    onec = nc.dram_tensor("onec", [1, 1], VW_DT, kind="ExternalInput").ap()
    out = nc.dram_tensor("out", [S, D], F32, kind="ExternalOutput").ap()

    with tile.TileContext(nc) as tc:
        consts = tc.alloc_tile_pool(name="consts", bufs=1)
        # weights as (128, KC, HD): partition=din%128, free=(din//128, dout)
        w_sb = {}
        for name, dram, dt_ in (("wv", wv, V_DT), ("wq", wq, QK_DT),
                                ("wk", wk, QK_DT)):
            t = consts.tile([128, KC, HD], dt_, tag=name)
            nc.sync.dma_start(out=t, in_=dram.rearrange("(c p) d -> p c d", p=128))
            w_sb[name] = t
        bias_sb = {}
        for name, dram in (("bq", bqv), ("bk", bkv), ("bv", bvv)):
            tA = consts.tile([128, 1], F32, tag=name + "A")
            nc.sync.dma_start(out=tA, in_=dram[0:128, :])
            tB = consts.tile([KS, 1], F32, tag=name + "B")
            nc.sync.dma_start(out=tB, in_=dram[128:HD, :])
            bias_sb[name] = (tA, tB)
        mb_sb = consts.tile([128, KT], F32, tag="mb")
        nc.sync.dma_start(out=mb_sb, in_=mb)
        woA = consts.tile([128, D], F32R, tag="woA")
        woB = consts.tile([KS, D], F32R, tag="woB")
        ident = consts.tile([128, 128], VW_DT, tag="ident")
        make_identity(nc, ident)

        # PE warm-up: dummy back-to-back matmuls so the HAM clock-gate sees a
        # busy PE while the first input chunks are still in flight; their
        # results are never read.
        with tc.tile_pool(name="warm_ps", bufs=1, space="PSUM") as wp:
            warm = wp.tile([128, 128], F32, tag="warm")
            for _ in range(24):
                nc.tensor.matmul(warm, ident, ident, start=True, stop=True)

        # persistent SBUF holding projected tensors
        proj = tc.alloc_tile_pool(name="proj", bufs=1)
        qA = proj.tile([128, S], F32R, tag="qA")  # heads 0,1 of qwT (scaled)
        qB = proj.tile([KS, S], F32R, tag="qB")  # head 2
        kA = proj.tile([128, S], F32R, tag="kA")
        kB = proj.tile([KS, S], F32R, tag="kB")
        # vw_ext[p, t, h, 0:64] = vw[t*128+p, 64h : 64h+64]; [..., 64] = 1.0
        vw_ext = proj.tile([128, KT, HPC, KS + 1], VW_DT, tag="vw")
        ones_bcast = bass.AP(
            tensor=onec.tensor, offset=0,
            ap=[[0, 128], [0, KT * HPC], [1, 1]],
        )
        nc.gpsimd.dma_start(out=vw_ext[:, :, :, KS:KS + 1], in_=ones_bcast)
        ctxA = proj.tile([128, S], F32R, tag="ctxA")  # heads 0,1 stacked
        ctxB = proj.tile([KS, S], F32R, tag="ctxB")   # head 2

        def head_slices(tileA, tileB):
            return [tileA[0:KS], tileA[KS:128], tileB[0:KS]]

        q_heads = head_slices(qA, qB)
        k_heads = head_slices(kA, kB)

        # ---- v pipeline first: with the v side in bf16 its DMA is small, and
        # doing it before q/k keeps PE order conflict-free (attention needs
        # vw_ext from the very first A@V).
        vwTA = proj.tile([128, S], VW_DT, tag="vwTA")
        vwTB = proj.tile([KS, S], VW_DT, tag="vwTB")
        # single input pool for v AND q/k tiles, allocated up front so the
        # q/k input DMAs never wait on v-phase compute via address reuse
        xin = tc.alloc_tile_pool(name="xin", bufs=6)
        bA, bB = bias_sb["bv"]
        with tc.tile_pool(name="vproj_ps", bufs=1, space="PSUM") as pp:
            psA = pp.tile([128, S], F32, tag="pA")
            psB = pp.tile([KS, S], F32, tag="pB")
            for c in range(KC):
                xt = xin.tile([128, S], V_DT, tag="xv", bufs=6)
                nc.sync.dma_start(out=xt, in_=vT[c * 128:(c + 1) * 128, :])
                for n in range(S // 512):
                    ns = slice(n * 512, (n + 1) * 512)
                    nc.tensor.matmul(
                        psA[:, ns], w_sb["wv"][:, c, 0:128], xt[:, ns],
                        start=(c == 0), stop=(c == KC - 1),
                    )
                for n in range(S // 512):
                    ns = slice(n * 512, (n + 1) * 512)
                    nc.tensor.matmul(
                        psB[:, ns], w_sb["wv"][:, c, 128:HD], xt[:, ns],
                        start=(c == 0), stop=(c == KC - 1),
                    )
            nc.vector.tensor_scalar_add(vwTA, psA, bA)
            nc.vector.tensor_scalar_add(vwTB, psB, bB)

        with tc.tile_pool(name="tr_ps", bufs=2, space="PSUM") as trp:
            for t in range(KT):
                ts_ = slice(t * 128, (t + 1) * 128)
                trA = trp.tile([128, 128], VW_DT, tag="tr")
                nc.tensor.transpose(trA, vwTA[:, ts_], ident)
                nc.vector.tensor_copy(
                    out=vw_ext[:, t, 0:2, 0:KS],
                    in_=trA.rearrange("p (h x) -> p h x", h=2),
                )
                trB = trp.tile([128, KS], VW_DT, tag="tr")
                nc.tensor.transpose(trB, vwTB[:, ts_], ident[0:KS, 0:KS])
                nc.vector.tensor_copy(out=vw_ext[:, t, 2, 0:KS], in_=trB)
        # ---- q/k projection: x (D,S) -> xwT (HD, S), full-width PSUM
        def project_qk(xT_dram, w_tile, biasAB, outA, outB, xin_pool, pp):
            psA = pp.tile([128, S], F32, tag="pA")
            psB = pp.tile([KS, S], F32, tag="pB")
            for c in range(KC):
                xt = xin_pool.tile([128, S], QK_DT, tag="x", bufs=8)
                nc.sync.dma_start(out=xt, in_=xT_dram[c * 128:(c + 1) * 128, :])
                for n in range(S // 512):
                    ns = slice(n * 512, (n + 1) * 512)
                    nc.tensor.matmul(
                        psA[:, ns], w_tile[:, c, 0:128], xt[:, ns],
                        start=(c == 0), stop=(c == KC - 1),
                    )
                for n in range(S // 512):
                    ns = slice(n * 512, (n + 1) * 512)
                    nc.tensor.matmul(
                        psB[:, ns], w_tile[:, c, 128:HD], xt[:, ns],
                        start=(c == 0), stop=(c == KC - 1),
                    )
            bA, bB = biasAB
            for e in range(2):
                es = slice(e * (S // 2), (e + 1) * (S // 2))
                nc.vector.tensor_scalar_add(outA[:, es], psA[:, es], bA)
                nc.vector.tensor_scalar_add(outB[:, es], psB[:, es], bB)

        with tc.tile_pool(name="qkproj_ps", bufs=1, space="PSUM") as pp:
            project_qk(qT, w_sb["wq"], bias_sb["bq"], qA, qB, xin, pp)
            project_qk(kT, w_sb["wk"], bias_sb["bk"], kA, kB, xin, pp)
        xin.release()
        nc.sync.dma_start(out=woA, in_=wo[0:128, :])
        nc.sync.dma_start(out=woB, in_=wo[128:HD, :])

        # ---- attention building blocks
        small = tc.alloc_tile_pool(name="small", bufs=1)
        psb = tc.alloc_tile_pool(name="p_sb", bufs=34)
        ob = tc.alloc_tile_pool(name="o_sb", bufs=3)
        sp = tc.alloc_tile_pool(name="s_ps", bufs=2, space="PSUM")

        def sexp_block(q0, h):
            """Scores + exp for one (slab, head): 16 key tiles -> p tiles."""
            qh_t, kh_t = q_heads[h], k_heads[h]
            p_tiles = []
            for t in range(KT):
                lhs_k = kh_t[:, t * 128:(t + 1) * 128]
                s_ps = sp.tile([128, QH], F32, tag="s")
                for j in range(2):
                    qs = slice(q0 + j * 512, q0 + (j + 1) * 512)
                    nc.tensor.matmul(
                        s_ps[:, j * 512:(j + 1) * 512], lhs_k, qh_t[:, qs],
                        start=True, stop=True,
                    )
                p_t = psb.tile([128, QH], VW_DT, tag="p")
                nc.scalar.activation(
                    out=p_t, in_=s_ps,
                    func=mybir.ActivationFunctionType.Exp,
                    bias=mb_sb[:, t:t + 1], scale=1.0,
                )
                p_tiles.append(p_t)
            return p_tiles

        # ---- main attention + interleaved output projection
        with tc.tile_pool(name="ctx_ps", bufs=2, space="PSUM") as ctxp:

            def av_block(q0, h, p_tiles):
                """A@V (+ denominator) accumulation and normalization."""
                ctx_ps = ctxp.tile([KS + 1, QH], F32, tag="ctx")
                for t in range(KT):
                    for j in range(2):
                        nc.tensor.matmul(
                            ctx_ps[:, j * 512:(j + 1) * 512],
                            vw_ext[:, t, h, :],
                            p_tiles[t][:, j * 512:(j + 1) * 512],
                            start=(t == 0), stop=(t == KT - 1),
                        )
                recip = small.tile([1, QH], F32, tag="recip")
                nc.vector.reciprocal(recip, ctx_ps[KS:KS + 1, :])
                rb = small.tile([KS, QH], F32, tag="rb")
                nc.gpsimd.partition_broadcast(rb, recip)
                dst = (ctxA[h * KS:(h + 1) * KS, q0:q0 + QH] if h < 2
                       else ctxB[:, q0:q0 + QH])
                nc.vector.tensor_mul(dst, ctx_ps[0:KS, :], rb)

            def out_proj(qt):
                # out[qt*128 : +128, :] = sum_h ctx[h, qt cols].T @ wo[h]
                # (borrows a (65, 1024) "ctx" PSUM slot)
                o_ps = ctxp.tile([128, D], F32, tag="ctx")
                qs = slice(qt * 128, (qt + 1) * 128)
                for noff, nsz in ((0, 512), (512, 256)):
                    nc.tensor.matmul(
                        o_ps[:, noff:noff + nsz],
                        ctxA[:, qs], woA[:, noff:noff + nsz],
                        start=True, stop=False,
                    )
                    nc.tensor.matmul(
                        o_ps[:, noff:noff + nsz],
                        ctxB[:, qs], woB[:, noff:noff + nsz],
                        start=False, stop=True,
                    )
                o_sb = ob.tile([128, D], F32, tag="o")
                nc.vector.tensor_copy(o_sb, o_ps)
                nc.sync.dma_start(out=out[qs, :], in_=o_sb)

            TPS = QH // 128  # out-proj tiles per slab
            pending = []
            for half in range(NSLAB):
                q0 = half * QH
                p0 = sexp_block(q0, 0)
                p1 = sexp_block(q0, 1)
                av_block(q0, 0, p0)
                for qt in pending[:TPS // 2]:
                    out_proj(qt)
                p2 = sexp_block(q0, 2)
                av_block(q0, 1, p1)
                for qt in pending[TPS // 2:]:
                    out_proj(qt)
                av_block(q0, 2, p2)
                pending = list(range(half * TPS, (half + 1) * TPS))
            for qt in pending:
                out_proj(qt)

        sp.release()
        ob.release()
        psb.release()
        small.release()
        proj.release()
        consts.release()

    nc.compile()
    return nc


_NC_CACHE = []


def _get_nc():
    if not _NC_CACHE:
        _NC_CACHE.append(build_program())
    return _NC_CACHE[0]


def _qk(a):
    return a.astype(NPBF) if QK_BF16 else a


def _vv(a):
    return a.astype(NPBF) if V_BF16 else a


def _prep_core_inputs(q, k, v, Wq, bq, Wk, bk, Wv, bv, Wo, v_mask):
    """Build the 8 per-core input maps (host-side sharding + layout)."""
    f = np.float32
    in_maps = []
    for b in range(B):
        l = max(float(np.asarray(v_mask[b], dtype=np.int64).sum()), 1.0)
        cb = np.log(l) / LOG_512 / np.sqrt(float(KS))
        qTb = _qk(np.ascontiguousarray(np.asarray(q[b], f).T))
        kTb = _qk(np.ascontiguousarray(np.asarray(k[b], f).T))
        vTb = _vv(np.ascontiguousarray(np.asarray(v[b], f).T))
        mbias = ((1.0 - np.asarray(v_mask[b], f)) * MASK_VALUE).astype(f)
        mb_t = np.ascontiguousarray(mbias.reshape(KT, 128).T)
        for g in range(4):
            sl = slice(g * HD, (g + 1) * HD)
            in_maps.append({
                "qT": qTb,
                "kT": kTb,
                "vT": vTb,
                "wq": _qk(np.ascontiguousarray(np.asarray(Wq, f)[:, sl] * cb)),
                "wk": _qk(np.ascontiguousarray(np.asarray(Wk, f)[:, sl])),
                "wv": _vv(np.ascontiguousarray(np.asarray(Wv, f)[:, sl])),
                "wo": np.ascontiguousarray(np.asarray(Wo, f)[sl, :]),
                "bqv": (np.asarray(bq, f)[sl] * cb).reshape(HD, 1).copy(),
                "bkv": np.asarray(bk, f)[sl].reshape(HD, 1).copy(),
                "bvv": np.asarray(bv, f)[sl].reshape(HD, 1).copy(),
                "mb": mb_t,
                "onec": np.ones((1, 1), NPBF if V_BF16 else f),
            })
    return in_maps


def kernel(q, k, v, Wq, bq, Wk, bk, Wv, bv, Wo, bo, v_mask, **_unused):
    nc = _get_nc()
    in_maps = _prep_core_inputs(q, k, v, Wq, bq, Wk, bk, Wv, bv, Wo, v_mask)
    res = run_bass_kernel_spmd(nc, in_maps, core_ids=list(range(N_CORES)))
    outs = [r["out"] for r in res.results]
    full = np.empty((B, S, D), np.float32)
    bo_f = np.asarray(bo, np.float32)
    for b in range(B):
        acc = outs[4 * b].copy()
        for g in range(1, 4):
            acc += outs[4 * b + g]
        full[b] = acc + bo_f
    return full


if __name__ == "__main__":
    np.random.seed(0)
    q = np.random.randn(B, S, D).astype(np.float32)
    k = np.random.randn(B, S, D).astype(np.float32)
    v = np.random.randn(B, S, D).astype(np.float32)
    Wq = np.random.randn(D, HEADS * KS).astype(np.float32) * 0.06
    Wk = np.random.randn(D, HEADS * KS).astype(np.float32) * 0.06
    Wv = np.random.randn(D, HEADS * KS).astype(np.float32) * 0.06
    Wo = np.random.randn(HEADS * KS, D).astype(np.float32) * 0.06
    z = np.zeros(HEADS * KS, np.float32)
    bo = np.zeros(D, np.float32)
    mask = np.random.randint(0, 2, (B, S)).astype(np.int32)
    o = kernel(q=q, k=k, v=v, Wq=Wq, bq=z, Wk=Wk, bk=z, Wv=Wv, bv=z,
               Wo=Wo, bo=bo, v_mask=mask)
    print(o.shape, o.dtype, float(np.abs(o).max()))
